# revision 18
# baseline (speedup 1.0000x reference)
"""MoE (16 routed experts, top-2, + shared expert) on 8 TRN2 NeuronCores.

Strategy (expert-parallel per the sharding hint):
  Host: router (x @ w_gate + bias, softmax, top-2, renormalize) — 0.1% of
    total FLOPs — plus the all-to-all dispatch: per-expert token gather into
    dense padded batches.  Experts are pair-balanced across cores (sorted
    pairing: core i gets the i-th largest and i-th smallest expert batch) so
    per-core padded work is near-uniform and minimal.
  Device (ONE SPMD launch, all 8 cores): core c runs the shared-expert
    SwiGLU FFN on its 2048-token slice, then the FFNs of its two routed
    experts on their gathered batches.  All matmul data travels bf16
    (PE runs bf16 at the same rate as fp32r, but DMA bytes halve); PSUM
    accumulation is fp32 and outputs are written fp32.
  Host: combine — scatter-add comb-weighted expert outputs + shared output.

Layout: all activations travel transposed (feature-major, token-minor).
Gate/up weights are host-packed per-ik ([IK*128, CK*128] with (ck, icol)
free order) so each 128-wide I-chunk loads as one contiguous DMA and the
first matmuls can start ~2us into the launch.  Weight DMAs ride SWDGE
(gpsimd) so their tile-recycle waits never block the x/y HWDGE queue.
"""

import numpy as np

# model dims (fixed for this problem)
E, TOPK, C, I = 16, 2, 768, 1536
B, T = 8, 2048
NCORE = 8
NTOK = B * T           # 16384
TPC = NTOK // NCORE    # 2048 tokens per core
CK = C // 128          # 6 contraction chunks for C
IK = I // 128          # 12 chunks for I
NBLK = 512             # token block = PE moving-dim per matmul

TRACE = False          # set True (from a driver) to capture NTFF timing
LAST = {}              # timing info from the most recent kernel() call

_progs = {}            # compiled program cache


def _bf16():
    import ml_dtypes

    return ml_dtypes.bfloat16


def _enable_axon_ntff_profiling():
    import sys
    import types

    if "antenv.axon_hooks" not in sys.modules:
        mod = types.ModuleType("antenv.axon_hooks")
        mod._hook = None
        mod.set_axon_ntff_profile_hook = lambda h: setattr(mod, "_hook", h)
        mod.get_axon_ntff_profile_hook = lambda: mod._hook
        sys.modules["antenv.axon_hooks"] = mod
    from antenv.axon_hooks import set_axon_ntff_profile_hook  # type: ignore
    from trn_agent_boot.trn_boot import _ntff_profile_via_ctypes

    set_axon_ntff_profile_hook(_ntff_profile_via_ctypes("/opt/axon/libaxon_pjrt.so"))
    import concourse.bass_utils as bu

    bu.upload_artifacts = lambda tmpdir: f"file://{tmpdir}"


def _blocks(m):
    out = []
    n0 = 0
    while n0 < m:
        nb = min(NBLK, m - n0)
        out.append((n0, nb))
        n0 += nb
    return out


def _pack_gu(wg, wu):
    """Gate+up [C, I] x2 -> [IK*128, 2*CK*128] bf16: row-block ik is one
    contiguous [128, 1536] DMA; free order (m, ck, icol) with m=gate|up."""
    s = np.stack([wg, wu])  # [2, C, I]
    p = s.reshape(2, CK, 128, IK, 128).transpose(3, 2, 0, 1, 4)
    p = p.reshape(IK * 128, 2 * CK * 128)
    return np.ascontiguousarray(p.astype(_bf16()))


def _pack_d(wd):
    """Down [I, C] -> [128, IK*C] bf16: one DMA; free order (ik, c)."""
    p = wd.reshape(IK, 128, C).transpose(1, 0, 2).reshape(128, IK * C)
    return np.ascontiguousarray(p.astype(_bf16()))


def _pack_x(xt):
    """[C, cap] bf16 -> [128, CK, cap]: one 3D DMA per token block."""
    return np.ascontiguousarray(xt.reshape(CK, 128, -1).transpose(1, 0, 2))


def _emit_ffn_phase(nc, tc, pools, aps, tag, cap, first=False):
    """Full SwiGLU FFN phase: y[C, cap] = down(silu(gate(x)) * up(x)).

    aps: dict with x (DRAM [C, cap] bf16), y (DRAM [C, cap] f32),
    wg/wu (DRAM [IK*128, CK*128] bf16 packed), wd (DRAM [I, C] bf16).
    """
    import concourse.mybir as mybir

    f32 = mybir.dt.float32
    bf16 = mybir.dt.bfloat16
    wpool, xpool, hpool, gpool, ypool, pgu, pd = (
        pools["w"],
        pools["x"],
        pools["h"],
        pools["g"],
        pools["y"],
        pools["pgu"],
        pools["pd"],
    )

    # weight tiles: per-ik fused gate+up [128, 2*CK*128]; down as one tile.
    # SWDGE so a blocked recycle-wait never stalls the x/y HWDGE queue.
    wgu_t = []
    for ik in range(IK):
        g = wpool.tile([128, 2 * CK * 128], bf16, tag=f"wgu{ik}")
        nc.gpsimd.dma_start(out=g[:], in_=aps["wgu"][ik * 128 : (ik + 1) * 128, :])
        wgu_t.append(g)
    wd_t = wpool.tile([128, IK * C], bf16, tag="wd")
    nc.gpsimd.dma_start(out=wd_t[:], in_=aps["wd"][:])

    def gate_ap(ik, ck):
        return wgu_t[ik][:, ck * 128 : (ck + 1) * 128]

    def up_ap(ik, ck):
        return wgu_t[ik][:, CK * 128 + ck * 128 : CK * 128 + (ck + 1) * 128]

    for n0, nblk in _blocks(cap):
        x_t = xpool.tile([128, CK, NBLK], bf16, tag="x")
        nc.sync.dma_start(out=x_t[:, :, :nblk], in_=aps["x"][:, :, n0 : n0 + nblk])
        h_t = hpool.tile([128, IK, NBLK], bf16, tag="h")
        for ik in range(IK):
            psg = pgu.tile([128, NBLK], f32, tag="psg")
            psu = pgu.tile([128, NBLK], f32, tag="psu")
            for ck in range(CK):
                nc.tensor.matmul(
                    psg[:, :nblk],
                    lhsT=gate_ap(ik, ck),
                    rhs=x_t[:, ck, :nblk],
                    start=(ck == 0),
                    stop=(ck == CK - 1),
                )
            for ck in range(CK):
                nc.tensor.matmul(
                    psu[:, :nblk],
                    lhsT=up_ap(ik, ck),
                    rhs=x_t[:, ck, :nblk],
                    start=(ck == 0),
                    stop=(ck == CK - 1),
                )
            ga = gpool.tile([128, NBLK], f32, tag="ga")
            nc.scalar.activation(
                ga[:, :nblk], psg[:, :nblk], mybir.ActivationFunctionType.Silu
            )
            nc.vector.tensor_mul(h_t[:, ik, :nblk], ga[:, :nblk], psu[:, :nblk])

        y_t = ypool.tile([128, CK, NBLK], f32, tag="y")
        for ck in range(CK):
            psd = pd.tile([128, NBLK], f32, tag="psd")
            for ik in range(IK):
                nc.tensor.matmul(
                    psd[:, :nblk],
                    lhsT=wd_t[:, ik * C + ck * 128 : ik * C + (ck + 1) * 128],
                    rhs=h_t[:, ik, :nblk],
                    start=(ik == 0),
                    stop=(ik == IK - 1),
                )
            nc.vector.tensor_copy(y_t[:, ck, :nblk], psd[:, :nblk])
            # per-ck store so the kernel tail only waits on the last chunk
            nc.sync.dma_start(
                out=aps["y"][:, ck, n0 : n0 + nblk], in_=y_t[:, ck, :nblk]
            )


def _build(cap_a, cap_b):
    """One launch: shared FFN (TPC tokens) + expert a (cap_a) + expert b."""
    from contextlib import ExitStack

    import concourse.tile as tile
    from concourse import bacc, mybir

    f32 = mybir.dt.float32
    bf16 = mybir.dt.bfloat16

    nc = bacc.Bacc("TRN2", target_bir_lowering=False, debug=False)
    phases = []
    for s, cap in (("s", TPC), ("a", cap_a), ("b", cap_b)):
        aps = {
            "x": nc.dram_tensor(
                f"x{s}", [128, CK, cap], bf16, kind="ExternalInput"
            ).ap(),
            "wgu": nc.dram_tensor(
                f"wgu{s}", [IK * 128, 2 * CK * 128], bf16, kind="ExternalInput"
            ).ap(),
            "wd": nc.dram_tensor(
                f"wd{s}", [128, IK * C], bf16, kind="ExternalInput"
            ).ap(),
            "y": nc.dram_tensor(
                f"y{s}", [128, CK, cap], f32, kind="ExternalOutput"
            ).ap(),
        }
        phases.append((aps, s, cap))

    with tile.TileContext(nc) as tc, ExitStack() as ctx:
        pools = {
            "w": ctx.enter_context(tc.tile_pool(name="wp", bufs=2)),
            "x": ctx.enter_context(tc.tile_pool(name="xp", bufs=2)),
            "h": ctx.enter_context(tc.tile_pool(name="hp", bufs=2)),
            "g": ctx.enter_context(tc.tile_pool(name="gp", bufs=2)),
            "y": ctx.enter_context(tc.tile_pool(name="yp", bufs=2)),
            "pgu": ctx.enter_context(tc.tile_pool(name="pgu", bufs=2, space="PSUM")),
            "pd": ctx.enter_context(tc.tile_pool(name="pd", bufs=2, space="PSUM")),
        }
        # HAM warmup: dep-free matmuls bridge the framework preamble and the
        # first weight/x DMA arrival (~7..14us) so the PE hits 2.4 GHz before
        # real work lands and the real MM stream starts dense.  Sized to end
        # right as supply lands: PE executes in order, so oversized warmup
        # delays real MMs.
        with tc.tile_pool(name="pw", bufs=2, space="PSUM") as pw:
            warm = pools["g"].tile([128, NBLK], bf16, tag="warm")
            nc.vector.memset(warm[:], 0.0)
            for _ in range(11):
                pw_t = pw.tile([128, NBLK], f32, tag="w")
                nc.tensor.matmul(
                    pw_t[:], lhsT=warm[:, :128], rhs=warm[:], start=True, stop=True
                )
        for pi, (aps, s, cap) in enumerate(phases):
            _emit_ffn_phase(nc, tc, pools, aps, s, cap, first=(pi == 0))

    nc.compile()
    return nc


def _run(nc, in_maps, tag):
    from concourse.bass_utils import run_bass_kernel_spmd

    if TRACE:
        _enable_axon_ntff_profiling()
        res = run_bass_kernel_spmd(nc, in_maps, list(range(NCORE)), trace=True)
        LAST[f"{tag}_ns"] = res.exec_time_ns
        if res.instructions_and_trace is not None:
            LAST[f"{tag}_trace"] = res.instructions_and_trace[1]
    else:
        res = run_bass_kernel_spmd(nc, in_maps, list(range(NCORE)), trace=False)
    return res.results


def _cap(n):
    # exact cap (any free-dim size works for matmul/DMA); floor for sanity
    return max(128, n)


def kernel(x, w_gate, expert_bias, wg, wu, wd, swg, swu, swd):
    LAST.clear()
    bf16 = _bf16()
    xf = np.ascontiguousarray(np.asarray(x, np.float32).reshape(NTOK, C))
    w_gate = np.asarray(w_gate, np.float32)
    expert_bias = np.asarray(expert_bias, np.float32)
    wg = np.asarray(wg, np.float32)
    wu = np.asarray(wu, np.float32)
    wd = np.asarray(wd, np.float32)

    # ---- host router: logits -> softmax -> top-2 -> renormalized weights
    logits = xf @ w_gate + expert_bias  # (N, E) f32
    m = logits.max(axis=1, keepdims=True)
    p = np.exp(logits - m, dtype=np.float32)
    p /= p.sum(axis=1, keepdims=True)
    top2 = np.argsort(-p, axis=1, kind="stable")[:, :TOPK]  # (N, 2)
    pv = np.take_along_axis(p, top2, axis=1)
    pv = pv / pv.sum(axis=1, keepdims=True)  # renormalized combine weights

    tok, wtok = [], []
    for e in range(E):
        sel0 = top2[:, 0] == e
        sel1 = top2[:, 1] == e
        ii = np.nonzero(sel0 | sel1)[0]
        ww = np.where(sel0, pv[:, 0], pv[:, 1])[ii].astype(np.float32)
        tok.append(ii)
        wtok.append(ww)
    counts = np.array([len(ii) for ii in tok])

    # ---- balanced pairing: core i gets (i-th largest, i-th smallest)
    order = np.argsort(-counts, kind="stable")
    slot_a = [int(order[i]) for i in range(NCORE)]
    slot_b = [int(order[E - 1 - i]) for i in range(NCORE)]
    cap_a = _cap(int(max(counts[e] for e in slot_a)))
    cap_b = _cap(int(max(counts[e] for e in slot_b)))

    key = (cap_a, cap_b)
    if key not in _progs:
        _progs[key] = _build(cap_a, cap_b)

    # ---- per-core inputs
    xf_bf = xf.astype(bf16)
    xt_bf = np.ascontiguousarray(xf_bf.T)  # (C, NTOK) bf16
    swgu_p = _pack_gu(np.asarray(swg, np.float32), np.asarray(swu, np.float32))
    swd_p = _pack_d(np.asarray(swd, np.float32))

    in_maps = []
    for c in range(NCORE):
        m_ = {
            "xs": _pack_x(xt_bf[:, c * TPC : (c + 1) * TPC]),
            "wgus": swgu_p,
            "wds": swd_p,
        }
        for s, e, cap in (("a", slot_a[c], cap_a), ("b", slot_b[c], cap_b)):
            ii = tok[e]
            xt = np.zeros((C, cap), bf16)
            xt[:, : len(ii)] = xf_bf[ii].T
            m_[f"x{s}"] = _pack_x(xt)
            m_[f"wgu{s}"] = _pack_gu(wg[e], wu[e])
            m_[f"wd{s}"] = _pack_d(wd[e])
        in_maps.append(m_)

    res = _run(_progs[key], in_maps, "launch")

    # ---- host combine: shared + scatter-add of comb-weighted expert outputs
    def unpack_y(y3, cap):
        # [128, CK, cap] -> (cap, C)
        return y3.transpose(2, 1, 0).reshape(cap, C)

    out = np.empty((NTOK, C), np.float32)
    for c in range(NCORE):
        out[c * TPC : (c + 1) * TPC] = unpack_y(res[c]["ys"], TPC)
    for s, slots, cap in (("a", slot_a, cap_a), ("b", slot_b, cap_b)):
        for c, e in enumerate(slots):
            ii = tok[e]
            y = unpack_y(res[c][f"y{s}"], cap)[: len(ii)]  # (len, C), unscaled
            out[ii] += y * wtok[e][:, None]

    if TRACE:
        LAST["total_ns"] = sum(
            v for k, v in LAST.items() if isinstance(v, int) and k.endswith("_ns")
        )
    return out.reshape(B, T, C)


# revision 21
# speedup vs baseline: 1.0917x; 1.0917x over previous
"""MoE (16 routed experts, top-2, + shared expert) on 8 TRN2 NeuronCores.

Strategy (expert-parallel per the sharding hint):
  Host: router (x @ w_gate + bias, softmax, top-2, renormalize) — 0.1% of
    total FLOPs — plus the all-to-all dispatch: per-expert token gather into
    dense padded batches.  Experts are pair-balanced across cores (sorted
    pairing: core i gets the i-th largest and i-th smallest expert batch) so
    per-core padded work is near-uniform and minimal.
  Device (ONE SPMD launch, all 8 cores): core c runs the shared-expert
    SwiGLU FFN on its 2048-token slice, then the FFNs of its two routed
    experts on their gathered batches.  All matmul data travels bf16
    (PE runs bf16 at the same rate as fp32r, but DMA bytes halve); PSUM
    accumulation is fp32 and outputs are written fp32.
  Host: combine — scatter-add comb-weighted expert outputs + shared output.

Layout: all activations travel transposed (feature-major, token-minor).
Gate/up weights are host-packed per-ik ([IK*128, CK*128] with (ck, icol)
free order) so each 128-wide I-chunk loads as one contiguous DMA and the
first matmuls can start ~2us into the launch.  Weight DMAs ride SWDGE
(gpsimd) so their tile-recycle waits never block the x/y HWDGE queue.
"""

import numpy as np

# model dims (fixed for this problem)
E, TOPK, C, I = 16, 2, 768, 1536
B, T = 8, 2048
NCORE = 8
NTOK = B * T           # 16384
TPC = NTOK // NCORE    # 2048 tokens per core
CK = C // 128          # 6 contraction chunks for C
IK = I // 128          # 12 chunks for I
NBLK = 512             # token block = PE moving-dim per matmul

TRACE = False          # set True (from a driver) to capture NTFF timing
LAST = {}              # timing info from the most recent kernel() call

_progs = {}            # compiled program cache


def _bf16():
    import ml_dtypes

    return ml_dtypes.bfloat16


def _enable_axon_ntff_profiling():
    import sys
    import types

    if "antenv.axon_hooks" not in sys.modules:
        mod = types.ModuleType("antenv.axon_hooks")
        mod._hook = None
        mod.set_axon_ntff_profile_hook = lambda h: setattr(mod, "_hook", h)
        mod.get_axon_ntff_profile_hook = lambda: mod._hook
        sys.modules["antenv.axon_hooks"] = mod
    from antenv.axon_hooks import set_axon_ntff_profile_hook  # type: ignore
    from trn_agent_boot.trn_boot import _ntff_profile_via_ctypes

    set_axon_ntff_profile_hook(_ntff_profile_via_ctypes("/opt/axon/libaxon_pjrt.so"))
    import concourse.bass_utils as bu

    bu.upload_artifacts = lambda tmpdir: f"file://{tmpdir}"


def _blocks(m):
    out = []
    n0 = 0
    while n0 < m:
        nb = min(NBLK, m - n0)
        out.append((n0, nb))
        n0 += nb
    return out


def _pack_gu(wg, wu):
    """Gate+up [C, I] x2 -> [IK*128, 2*CK*128] bf16: row-block ik is one
    contiguous [128, 1536] DMA; free order (m, ck, icol) with m=gate|up."""
    s = np.stack([wg, wu])  # [2, C, I]
    p = s.reshape(2, CK, 128, IK, 128).transpose(3, 2, 0, 1, 4)
    p = p.reshape(IK * 128, 2 * CK * 128)
    return np.ascontiguousarray(p.astype(_bf16()))


def _pack_d(wd):
    """Down [I, C] -> [128, IK*C] bf16: one DMA; free order (ik, c)."""
    p = wd.reshape(IK, 128, C).transpose(1, 0, 2).reshape(128, IK * C)
    return np.ascontiguousarray(p.astype(_bf16()))


def _pack_x(xt):
    """[C, cap] bf16 -> [128, CK, cap]: one 3D DMA per token block."""
    return np.ascontiguousarray(xt.reshape(CK, 128, -1).transpose(1, 0, 2))


def _emit_ffn_phase(nc, tc, pools, aps, tag, cap):
    """Full SwiGLU FFN phase: y[C, cap] = down(silu(gate(x)) * up(x)).

    aps: dict with x (DRAM [C, cap] bf16), y (DRAM [C, cap] f32),
    wg/wu (DRAM [IK*128, CK*128] bf16 packed), wd (DRAM [I, C] bf16).
    """
    import concourse.mybir as mybir

    f32 = mybir.dt.float32
    bf16 = mybir.dt.bfloat16
    wpool, xpool, hpool, gpool, ypool, pgu, pd = (
        pools["w"],
        pools["x"],
        pools["h"],
        pools["g"],
        pools["y"],
        pools["pgu"],
        pools["pd"],
    )

    # weight tiles: per-ik fused gate+up [128, 2*CK*128]; down as one tile.
    # SWDGE so a blocked recycle-wait never stalls the x/y HWDGE queue.
    wgu_t = []
    for ik in range(IK):
        g = wpool.tile([128, 2 * CK * 128], bf16, tag=f"wgu{ik}")
        nc.gpsimd.dma_start(out=g[:], in_=aps["wgu"][ik * 128 : (ik + 1) * 128, :])
        wgu_t.append(g)
    wd_t = wpool.tile([128, IK * C], bf16, tag="wd")
    nc.gpsimd.dma_start(out=wd_t[:], in_=aps["wd"][:])

    def gate_ap(ik, ck):
        return wgu_t[ik][:, ck * 128 : (ck + 1) * 128]

    def up_ap(ik, ck):
        return wgu_t[ik][:, CK * 128 + ck * 128 : CK * 128 + (ck + 1) * 128]

    for n0, nblk in _blocks(cap):
        x_t = xpool.tile([128, CK, NBLK], bf16, tag="x")
        nc.sync.dma_start(out=x_t[:, :, :nblk], in_=aps["x"][:, :, n0 : n0 + nblk])
        h_t = hpool.tile([128, IK, NBLK], bf16, tag="h")
        for ik in range(IK):
            psg = pgu.tile([128, NBLK], f32, tag="psg")
            psu = pgu.tile([128, NBLK], f32, tag="psu")
            for ck in range(CK):
                nc.tensor.matmul(
                    psg[:, :nblk],
                    lhsT=gate_ap(ik, ck),
                    rhs=x_t[:, ck, :nblk],
                    start=(ck == 0),
                    stop=(ck == CK - 1),
                )
            for ck in range(CK):
                nc.tensor.matmul(
                    psu[:, :nblk],
                    lhsT=up_ap(ik, ck),
                    rhs=x_t[:, ck, :nblk],
                    start=(ck == 0),
                    stop=(ck == CK - 1),
                )
            ga = gpool.tile([128, NBLK], f32, tag="ga")
            nc.scalar.activation(
                ga[:, :nblk], psg[:, :nblk], mybir.ActivationFunctionType.Silu
            )
            nc.vector.tensor_mul(h_t[:, ik, :nblk], ga[:, :nblk], psu[:, :nblk])

        y_t = ypool.tile([128, CK, NBLK], f32, tag="y")
        for ck in range(CK):
            psd = pd.tile([128, NBLK], f32, tag="psd")
            for ik in range(IK):
                nc.tensor.matmul(
                    psd[:, :nblk],
                    lhsT=wd_t[:, ik * C + ck * 128 : ik * C + (ck + 1) * 128],
                    rhs=h_t[:, ik, :nblk],
                    start=(ik == 0),
                    stop=(ik == IK - 1),
                )
            nc.vector.tensor_copy(y_t[:, ck, :nblk], psd[:, :nblk])
            # per-ck store so the kernel tail only waits on the last chunk
            nc.sync.dma_start(
                out=aps["y"][:, ck, n0 : n0 + nblk], in_=y_t[:, ck, :nblk]
            )


def _build(cap_a, cap_b):
    """One launch: shared FFN (TPC tokens) + expert a (cap_a) + expert b."""
    from contextlib import ExitStack

    import concourse.tile as tile
    from concourse import bacc, mybir

    f32 = mybir.dt.float32
    bf16 = mybir.dt.bfloat16

    nc = bacc.Bacc("TRN2", target_bir_lowering=False, debug=False)
    phases = []
    for s, cap in (("s", TPC), ("a", cap_a), ("b", cap_b)):
        aps = {
            "x": nc.dram_tensor(
                f"x{s}", [128, CK, cap], bf16, kind="ExternalInput"
            ).ap(),
            "wgu": nc.dram_tensor(
                f"wgu{s}", [IK * 128, 2 * CK * 128], bf16, kind="ExternalInput"
            ).ap(),
            "wd": nc.dram_tensor(
                f"wd{s}", [128, IK * C], bf16, kind="ExternalInput"
            ).ap(),
            "y": nc.dram_tensor(
                f"y{s}", [128, CK, cap], f32, kind="ExternalOutput"
            ).ap(),
        }
        phases.append((aps, s, cap))

    with tile.TileContext(nc) as tc, ExitStack() as ctx:
        pools = {
            "w": ctx.enter_context(tc.tile_pool(name="wp", bufs=2)),
            "x": ctx.enter_context(tc.tile_pool(name="xp", bufs=2)),
            "h": ctx.enter_context(tc.tile_pool(name="hp", bufs=2)),
            "g": ctx.enter_context(tc.tile_pool(name="gp", bufs=2)),
            "y": ctx.enter_context(tc.tile_pool(name="yp", bufs=2)),
            "pgu": ctx.enter_context(tc.tile_pool(name="pgu", bufs=2, space="PSUM")),
            "pd": ctx.enter_context(tc.tile_pool(name="pd", bufs=2, space="PSUM")),
        }
        # HAM warmup: dep-free matmuls bridge the framework preamble and the
        # first weight/x DMA arrival (~7..14us) so the PE hits 2.4 GHz before
        # real work lands and the real MM stream starts dense.  Sized to end
        # right as supply lands: PE executes in order, so oversized warmup
        # delays real MMs.
        with tc.tile_pool(name="pw", bufs=2, space="PSUM") as pw:
            warm = pools["g"].tile([128, NBLK], bf16, tag="warm")
            nc.vector.memset(warm[:], 0.0)
            for _ in range(15):
                pw_t = pw.tile([128, NBLK], f32, tag="w")
                nc.tensor.matmul(
                    pw_t[:], lhsT=warm[:, :128], rhs=warm[:], start=True, stop=True
                )
        for aps, s, cap in phases:
            _emit_ffn_phase(nc, tc, pools, aps, s, cap)

    nc.compile()
    return nc


def _run(nc, in_maps, tag):
    from concourse.bass_utils import run_bass_kernel_spmd

    if TRACE:
        _enable_axon_ntff_profiling()
        res = run_bass_kernel_spmd(nc, in_maps, list(range(NCORE)), trace=True)
        LAST[f"{tag}_ns"] = res.exec_time_ns
        if res.instructions_and_trace is not None:
            LAST[f"{tag}_trace"] = res.instructions_and_trace[1]
    else:
        res = run_bass_kernel_spmd(nc, in_maps, list(range(NCORE)), trace=False)
    return res.results


def _cap(n):
    # exact cap (any free-dim size works for matmul/DMA); floor for sanity
    return max(128, n)


def kernel(x, w_gate, expert_bias, wg, wu, wd, swg, swu, swd):
    LAST.clear()
    bf16 = _bf16()
    xf = np.ascontiguousarray(np.asarray(x, np.float32).reshape(NTOK, C))
    w_gate = np.asarray(w_gate, np.float32)
    expert_bias = np.asarray(expert_bias, np.float32)
    wg = np.asarray(wg, np.float32)
    wu = np.asarray(wu, np.float32)
    wd = np.asarray(wd, np.float32)

    # ---- host router: logits -> softmax -> top-2 -> renormalized weights
    logits = xf @ w_gate + expert_bias  # (N, E) f32
    m = logits.max(axis=1, keepdims=True)
    p = np.exp(logits - m, dtype=np.float32)
    p /= p.sum(axis=1, keepdims=True)
    top2 = np.argsort(-p, axis=1, kind="stable")[:, :TOPK]  # (N, 2)
    pv = np.take_along_axis(p, top2, axis=1)
    pv = pv / pv.sum(axis=1, keepdims=True)  # renormalized combine weights

    tok, wtok = [], []
    for e in range(E):
        sel0 = top2[:, 0] == e
        sel1 = top2[:, 1] == e
        ii = np.nonzero(sel0 | sel1)[0]
        ww = np.where(sel0, pv[:, 0], pv[:, 1])[ii].astype(np.float32)
        tok.append(ii)
        wtok.append(ww)
    counts = np.array([len(ii) for ii in tok])

    # ---- balanced pairing: core i gets (i-th largest, i-th smallest)
    order = np.argsort(-counts, kind="stable")
    slot_a = [int(order[i]) for i in range(NCORE)]
    slot_b = [int(order[E - 1 - i]) for i in range(NCORE)]
    cap_a = _cap(int(max(counts[e] for e in slot_a)))
    cap_b = _cap(int(max(counts[e] for e in slot_b)))

    key = (cap_a, cap_b)
    if key not in _progs:
        _progs[key] = _build(cap_a, cap_b)

    # ---- per-core inputs
    xf_bf = xf.astype(bf16)
    xt_bf = np.ascontiguousarray(xf_bf.T)  # (C, NTOK) bf16
    swgu_p = _pack_gu(np.asarray(swg, np.float32), np.asarray(swu, np.float32))
    swd_p = _pack_d(np.asarray(swd, np.float32))

    in_maps = []
    for c in range(NCORE):
        m_ = {
            "xs": _pack_x(xt_bf[:, c * TPC : (c + 1) * TPC]),
            "wgus": swgu_p,
            "wds": swd_p,
        }
        for s, e, cap in (("a", slot_a[c], cap_a), ("b", slot_b[c], cap_b)):
            ii = tok[e]
            xt = np.zeros((C, cap), bf16)
            xt[:, : len(ii)] = xf_bf[ii].T
            m_[f"x{s}"] = _pack_x(xt)
            m_[f"wgu{s}"] = _pack_gu(wg[e], wu[e])
            m_[f"wd{s}"] = _pack_d(wd[e])
        in_maps.append(m_)

    res = _run(_progs[key], in_maps, "launch")

    # ---- host combine: shared + scatter-add of comb-weighted expert outputs
    def unpack_y(y3, cap):
        # [128, CK, cap] -> (cap, C)
        return y3.transpose(2, 1, 0).reshape(cap, C)

    out = np.empty((NTOK, C), np.float32)
    for c in range(NCORE):
        out[c * TPC : (c + 1) * TPC] = unpack_y(res[c]["ys"], TPC)
    for s, slots, cap in (("a", slot_a, cap_a), ("b", slot_b, cap_b)):
        for c, e in enumerate(slots):
            ii = tok[e]
            y = unpack_y(res[c][f"y{s}"], cap)[: len(ii)]  # (len, C), unscaled
            out[ii] += y * wtok[e][:, None]

    if TRACE:
        LAST["total_ns"] = sum(
            v for k, v in LAST.items() if isinstance(v, int) and k.endswith("_ns")
        )
    return out.reshape(B, T, C)


# revision 24
# speedup vs baseline: 1.1390x; 1.0433x over previous
"""MoE (16 routed experts, top-2, + shared expert) on 8 TRN2 NeuronCores.

Strategy (expert-parallel per the sharding hint):
  Host: router (x @ w_gate + bias, softmax, top-2, renormalize) — 0.1% of
    total FLOPs — plus the all-to-all dispatch: per-expert token gather into
    dense padded batches.  Experts are pair-balanced across cores (sorted
    pairing: core i gets the i-th largest and i-th smallest expert batch) so
    per-core padded work is near-uniform and minimal.
  Device (ONE SPMD launch, all 8 cores): core c runs the shared-expert
    SwiGLU FFN on its 2048-token slice, then the FFNs of its two routed
    experts on their gathered batches.  All matmul data travels bf16
    (PE runs bf16 at the same rate as fp32r, but DMA bytes halve); PSUM
    accumulation is fp32 and outputs are written fp32.
  Host: combine — scatter-add comb-weighted expert outputs + shared output.

Layout: all activations travel transposed (feature-major, token-minor).
Gate/up weights are host-packed per-ik ([IK*128, CK*128] with (ck, icol)
free order) so each 128-wide I-chunk loads as one contiguous DMA and the
first matmuls can start ~2us into the launch.  Weight DMAs ride SWDGE
(gpsimd) so their tile-recycle waits never block the x/y HWDGE queue.
"""

import numpy as np

# model dims (fixed for this problem)
E, TOPK, C, I = 16, 2, 768, 1536
B, T = 8, 2048
NCORE = 8
NTOK = B * T           # 16384
TPC = NTOK // NCORE    # 2048 tokens per core
CK = C // 128          # 6 contraction chunks for C
IK = I // 128          # 12 chunks for I
NBLK = 512             # token block = PE moving-dim per matmul

TRACE = False          # set True (from a driver) to capture NTFF timing
LAST = {}              # timing info from the most recent kernel() call

_progs = {}            # compiled program cache


def _bf16():
    import ml_dtypes

    return ml_dtypes.bfloat16


def _enable_axon_ntff_profiling():
    import sys
    import types

    if "antenv.axon_hooks" not in sys.modules:
        mod = types.ModuleType("antenv.axon_hooks")
        mod._hook = None
        mod.set_axon_ntff_profile_hook = lambda h: setattr(mod, "_hook", h)
        mod.get_axon_ntff_profile_hook = lambda: mod._hook
        sys.modules["antenv.axon_hooks"] = mod
    from antenv.axon_hooks import set_axon_ntff_profile_hook  # type: ignore
    from trn_agent_boot.trn_boot import _ntff_profile_via_ctypes

    set_axon_ntff_profile_hook(_ntff_profile_via_ctypes("/opt/axon/libaxon_pjrt.so"))
    import concourse.bass_utils as bu

    bu.upload_artifacts = lambda tmpdir: f"file://{tmpdir}"


def _blocks(m):
    out = []
    n0 = 0
    while n0 < m:
        nb = min(NBLK, m - n0)
        out.append((n0, nb))
        n0 += nb
    return out


def _pack_gu(wg, wu):
    """Gate+up [C, I] x2 -> [IK*128, 2*CK*128] bf16: row-block ik is one
    contiguous [128, 1536] DMA; free order (m, ck, icol) with m=gate|up."""
    s = np.stack([wg, wu])  # [2, C, I]
    p = s.reshape(2, CK, 128, IK, 128).transpose(3, 2, 0, 1, 4)
    p = p.reshape(IK * 128, 2 * CK * 128)
    return np.ascontiguousarray(p.astype(_bf16()))


def _pack_d(wd):
    """Down [I, C] -> [128, IK*C] bf16: one DMA; free order (ik, c)."""
    p = wd.reshape(IK, 128, C).transpose(1, 0, 2).reshape(128, IK * C)
    return np.ascontiguousarray(p.astype(_bf16()))


def _pack_x(xt):
    """[C, cap] bf16 -> [128, CK, cap]: one 3D DMA per token block."""
    return np.ascontiguousarray(xt.reshape(CK, 128, -1).transpose(1, 0, 2))


def _emit_ffn_phase(nc, tc, pools, aps, tag, cap):
    """Full SwiGLU FFN phase: y[C, cap] = down(silu(gate(x)) * up(x)).

    aps: dict with x (DRAM [C, cap] bf16), y (DRAM [C, cap] f32),
    wg/wu (DRAM [IK*128, CK*128] bf16 packed), wd (DRAM [I, C] bf16).
    """
    import concourse.mybir as mybir

    f32 = mybir.dt.float32
    bf16 = mybir.dt.bfloat16
    wpool, xpool, hpool, gpool, ypool, pgu, pd = (
        pools["w"],
        pools["x"],
        pools["h"],
        pools["g"],
        pools["y"],
        pools["pgu"],
        pools["pd"],
    )

    # weight tiles: per-ik fused gate+up [128, 2*CK*128]; down as one tile.
    # SWDGE so a blocked recycle-wait never stalls the x/y HWDGE queue.
    wgu_t = []
    for ik in range(IK):
        g = wpool.tile([128, 2 * CK * 128], bf16, tag=f"wgu{ik}")
        nc.gpsimd.dma_start(out=g[:], in_=aps["wgu"][ik * 128 : (ik + 1) * 128, :])
        wgu_t.append(g)
    wd_t = wpool.tile([128, IK * C], bf16, tag="wd")
    nc.gpsimd.dma_start(out=wd_t[:], in_=aps["wd"][:])

    def gate_ap(ik, ck):
        return wgu_t[ik][:, ck * 128 : (ck + 1) * 128]

    def up_ap(ik, ck):
        return wgu_t[ik][:, CK * 128 + ck * 128 : CK * 128 + (ck + 1) * 128]

    for n0, nblk in _blocks(cap):
        x_t = xpool.tile([128, CK, NBLK], bf16, tag="x")
        nc.sync.dma_start(out=x_t[:, :, :nblk], in_=aps["x"][:, :, n0 : n0 + nblk])
        h_t = hpool.tile([128, IK, NBLK], bf16, tag="h")
        for ik in range(IK):
            psg = pgu.tile([128, NBLK], f32, tag="psg")
            psu = pgu.tile([128, NBLK], f32, tag="psu")
            for ck in range(CK):
                nc.tensor.matmul(
                    psg[:, :nblk],
                    lhsT=gate_ap(ik, ck),
                    rhs=x_t[:, ck, :nblk],
                    start=(ck == 0),
                    stop=(ck == CK - 1),
                )
            for ck in range(CK):
                nc.tensor.matmul(
                    psu[:, :nblk],
                    lhsT=up_ap(ik, ck),
                    rhs=x_t[:, ck, :nblk],
                    start=(ck == 0),
                    stop=(ck == CK - 1),
                )
            ga = gpool.tile([128, NBLK], f32, tag="ga")
            nc.scalar.activation(
                ga[:, :nblk], psg[:, :nblk], mybir.ActivationFunctionType.Silu
            )
            nc.vector.tensor_mul(h_t[:, ik, :nblk], ga[:, :nblk], psu[:, :nblk])

        y_t = ypool.tile([128, CK, NBLK], bf16, tag="y")
        for ck in range(CK):
            psd = pd.tile([128, NBLK], f32, tag="psd")
            for ik in range(IK):
                nc.tensor.matmul(
                    psd[:, :nblk],
                    lhsT=wd_t[:, ik * C + ck * 128 : ik * C + (ck + 1) * 128],
                    rhs=h_t[:, ik, :nblk],
                    start=(ik == 0),
                    stop=(ik == IK - 1),
                )
            nc.vector.tensor_copy(y_t[:, ck, :nblk], psd[:, :nblk])
            # per-ck store so the kernel tail only waits on the last chunk
            nc.sync.dma_start(
                out=aps["y"][:, ck, n0 : n0 + nblk], in_=y_t[:, ck, :nblk]
            )


def _build(cap_a, cap_b):
    """One launch: shared FFN (TPC tokens) + expert a (cap_a) + expert b."""
    from contextlib import ExitStack

    import concourse.tile as tile
    from concourse import bacc, mybir

    f32 = mybir.dt.float32
    bf16 = mybir.dt.bfloat16

    nc = bacc.Bacc("TRN2", target_bir_lowering=False, debug=False)
    phases = []
    for s, cap in (("s", TPC), ("a", cap_a), ("b", cap_b)):
        aps = {
            "x": nc.dram_tensor(
                f"x{s}", [128, CK, cap], bf16, kind="ExternalInput"
            ).ap(),
            "wgu": nc.dram_tensor(
                f"wgu{s}", [IK * 128, 2 * CK * 128], bf16, kind="ExternalInput"
            ).ap(),
            "wd": nc.dram_tensor(
                f"wd{s}", [128, IK * C], bf16, kind="ExternalInput"
            ).ap(),
            "y": nc.dram_tensor(
                f"y{s}", [128, CK, cap], bf16, kind="ExternalOutput"
            ).ap(),
        }
        phases.append((aps, s, cap))

    with tile.TileContext(nc) as tc, ExitStack() as ctx:
        pools = {
            "w": ctx.enter_context(tc.tile_pool(name="wp", bufs=2)),
            "x": ctx.enter_context(tc.tile_pool(name="xp", bufs=2)),
            "h": ctx.enter_context(tc.tile_pool(name="hp", bufs=2)),
            "g": ctx.enter_context(tc.tile_pool(name="gp", bufs=2)),
            "y": ctx.enter_context(tc.tile_pool(name="yp", bufs=2)),
            "pgu": ctx.enter_context(tc.tile_pool(name="pgu", bufs=2, space="PSUM")),
            "pd": ctx.enter_context(tc.tile_pool(name="pd", bufs=2, space="PSUM")),
        }
        # HAM warmup: dep-free matmuls bridge the framework preamble and the
        # first weight/x DMA arrival (~7..14us) so the PE hits 2.4 GHz before
        # real work lands and the real MM stream starts dense.  Sized to end
        # right as supply lands: PE executes in order, so oversized warmup
        # delays real MMs.
        with tc.tile_pool(name="pw", bufs=2, space="PSUM") as pw:
            warm = pools["g"].tile([128, NBLK], bf16, tag="warm")
            nc.vector.memset(warm[:], 0.0)
            for _ in range(15):
                pw_t = pw.tile([128, NBLK], f32, tag="w")
                nc.tensor.matmul(
                    pw_t[:], lhsT=warm[:, :128], rhs=warm[:], start=True, stop=True
                )
        for aps, s, cap in phases:
            _emit_ffn_phase(nc, tc, pools, aps, s, cap)

    nc.compile()
    return nc


def _run(nc, in_maps, tag):
    from concourse.bass_utils import run_bass_kernel_spmd

    if TRACE:
        _enable_axon_ntff_profiling()
        res = run_bass_kernel_spmd(nc, in_maps, list(range(NCORE)), trace=True)
        LAST[f"{tag}_ns"] = res.exec_time_ns
        if res.instructions_and_trace is not None:
            LAST[f"{tag}_trace"] = res.instructions_and_trace[1]
    else:
        res = run_bass_kernel_spmd(nc, in_maps, list(range(NCORE)), trace=False)
    return res.results


def _cap(n):
    # exact cap (any free-dim size works for matmul/DMA); floor for sanity
    return max(128, n)


def kernel(x, w_gate, expert_bias, wg, wu, wd, swg, swu, swd):
    LAST.clear()
    bf16 = _bf16()
    xf = np.ascontiguousarray(np.asarray(x, np.float32).reshape(NTOK, C))
    w_gate = np.asarray(w_gate, np.float32)
    expert_bias = np.asarray(expert_bias, np.float32)
    wg = np.asarray(wg, np.float32)
    wu = np.asarray(wu, np.float32)
    wd = np.asarray(wd, np.float32)

    # ---- host router: logits -> softmax -> top-2 -> renormalized weights
    logits = xf @ w_gate + expert_bias  # (N, E) f32
    m = logits.max(axis=1, keepdims=True)
    p = np.exp(logits - m, dtype=np.float32)
    p /= p.sum(axis=1, keepdims=True)
    top2 = np.argsort(-p, axis=1, kind="stable")[:, :TOPK]  # (N, 2)
    pv = np.take_along_axis(p, top2, axis=1)
    pv = pv / pv.sum(axis=1, keepdims=True)  # renormalized combine weights

    tok, wtok = [], []
    for e in range(E):
        sel0 = top2[:, 0] == e
        sel1 = top2[:, 1] == e
        ii = np.nonzero(sel0 | sel1)[0]
        ww = np.where(sel0, pv[:, 0], pv[:, 1])[ii].astype(np.float32)
        tok.append(ii)
        wtok.append(ww)
    counts = np.array([len(ii) for ii in tok])

    # ---- balanced pairing: core i gets (i-th largest, i-th smallest)
    order = np.argsort(-counts, kind="stable")
    slot_a = [int(order[i]) for i in range(NCORE)]
    slot_b = [int(order[E - 1 - i]) for i in range(NCORE)]
    cap_a = _cap(int(max(counts[e] for e in slot_a)))
    cap_b = _cap(int(max(counts[e] for e in slot_b)))

    key = (cap_a, cap_b)
    if key not in _progs:
        _progs[key] = _build(cap_a, cap_b)

    # ---- per-core inputs
    xf_bf = xf.astype(bf16)
    xt_bf = np.ascontiguousarray(xf_bf.T)  # (C, NTOK) bf16
    swgu_p = _pack_gu(np.asarray(swg, np.float32), np.asarray(swu, np.float32))
    swd_p = _pack_d(np.asarray(swd, np.float32))

    in_maps = []
    for c in range(NCORE):
        m_ = {
            "xs": _pack_x(xt_bf[:, c * TPC : (c + 1) * TPC]),
            "wgus": swgu_p,
            "wds": swd_p,
        }
        for s, e, cap in (("a", slot_a[c], cap_a), ("b", slot_b[c], cap_b)):
            ii = tok[e]
            xt = np.zeros((C, cap), bf16)
            xt[:, : len(ii)] = xf_bf[ii].T
            m_[f"x{s}"] = _pack_x(xt)
            m_[f"wgu{s}"] = _pack_gu(wg[e], wu[e])
            m_[f"wd{s}"] = _pack_d(wd[e])
        in_maps.append(m_)

    res = _run(_progs[key], in_maps, "launch")

    # ---- host combine: shared + scatter-add of comb-weighted expert outputs
    def unpack_y(y3, cap):
        # [128, CK, cap] bf16 -> (cap, C) f32
        return y3.transpose(2, 1, 0).reshape(cap, C).astype(np.float32)

    out = np.empty((NTOK, C), np.float32)
    for c in range(NCORE):
        out[c * TPC : (c + 1) * TPC] = unpack_y(res[c]["ys"], TPC)
    for s, slots, cap in (("a", slot_a, cap_a), ("b", slot_b, cap_b)):
        for c, e in enumerate(slots):
            ii = tok[e]
            y = unpack_y(res[c][f"y{s}"], cap)[: len(ii)]  # (len, C), unscaled
            out[ii] += y * wtok[e][:, None]

    if TRACE:
        LAST["total_ns"] = sum(
            v for k, v in LAST.items() if isinstance(v, int) and k.endswith("_ns")
        )
    return out.reshape(B, T, C)
